# revision 3
# baseline (speedup 1.0000x reference)
"""Distributed Trainium2 kernel: LayerNorm + 16-head attention + out-proj, v6.

Sharding: (head x batch). Cores 0-3 own batch 0, cores 4-7 batch 1; core g
owns the 4 global heads {4*(g%4) .. 4*(g%4)+3} of its batch. Each core
LayerNorms/transposes only its batch half (2048 tokens), computes q/k/v for
its 4 heads, runs attention, then a 4-core AllToAll per head-pair
redistributes sa (+softmax denominators) so core g out-projects flat token
rows [g*512, (g+1)*512). The host passes per-core x halves and weight column
slices, so the SPMD program needs no core-dependent addressing.

Engine plan: PE does transposes/QKV/S/PV/out-proj back-to-back (bf16
stationaries -> FWL); ScalarE does only exp in the attention phase; GpSimd
applies LayerNorm during the f32->bf16 cast (per-partition scale/bias);
VectorE does stats + PSUM evacuations; softmax normalization is deferred
past the AllToAll into the out-proj phase (rank-2 broadcast matmuls).
PSUM: s 2x[128,1024] + qkv/out accs 2x[128,512] + misc/sa 2x[128,512].
"""
import numpy as np
import ml_dtypes

import concourse.bass as bass
import concourse.mybir as mybir
import concourse.tile as tile
from concourse import bacc
from concourse.bass_utils import run_bass_kernel_spmd

F32 = mybir.dt.float32
F32R = mybir.dt.float32r
BF16 = mybir.dt.bfloat16
AF = mybir.ActivationFunctionType
OP = mybir.AluOpType

B = 2
N = 2048
D = 1024
DH = 64
SCALE = 0.125
EPS = 1e-5

NT = B * N              # 4096 flat tokens
P = 128
NB = N                  # 2048 batch-local tokens per core
NBLK = NB // 512        # 4 blocks
MT = NB // P            # 16 key tiles
DC = D // P             # 8 contraction chunks
H_LOC = 4               # heads per core (2 pairs)
QKV_COLS = 3 * H_LOC * DH   # 768 local qkv cols (6 groups of 128)
TOK_OUT = NT // 8       # 512 output rows per core


def _build(with_qkv_bias):
    nc = bacc.Bacc("TRN2", target_bir_lowering=False, debug=False, num_devices=8)

    x_ext = nc.dram_tensor("x", [NB, D], F32, kind="ExternalInput")
    wqkv_ext = nc.dram_tensor("wqkv", [D, QKV_COLS], BF16, kind="ExternalInput")
    bqkv_ext = nc.dram_tensor("bqkv", [QKV_COLS, 1], F32, kind="ExternalInput")
    wout_ext = nc.dram_tensor("wout", [D, D], BF16, kind="ExternalInput")
    bout_ext = nc.dram_tensor("bout", [1, D], F32, kind="ExternalInput")
    id_ext = nc.dram_tensor("ident", [P, P], F32, kind="ExternalInput")
    zsel_ext = nc.dram_tensor("zsel", [8, P], F32, kind="ExternalInput")
    out_ext = nc.dram_tensor("out", [TOK_OUT, D], F32, kind="ExternalOutput")

    with tile.TileContext(nc) as tc:
        with tc.tile_pool(name="persist", bufs=1) as pp, \
             tc.tile_pool(name="xs", bufs=2) as xsp, \
             tc.tile_pool(name="xb", bufs=2) as xbp, \
             tc.tile_pool(name="es", bufs=17) as esp, \
             tc.tile_pool(name="sm", bufs=2) as smp, \
             tc.tile_pool(name="dram", bufs=1, space="DRAM") as dram, \
             tc.tile_pool(name="ps_s", bufs=2, space="PSUM") as ps_s, \
             tc.tile_pool(name="ps_q", bufs=2, space="PSUM") as ps_q, \
             tc.tile_pool(name="ps_m", bufs=2, space="PSUM") as ps_m:

            # ---- constants / weights -------------------------------------
            identf = pp.tile([P, P], F32, tag="identf")
            nc.gpsimd.dma_start(identf[:], id_ext.ap())
            identb = pp.tile([P, P], BF16, tag="identb")
            nc.vector.tensor_copy(identb[:], identf[:])

            onesp_32 = pp.tile([P, 1], F32, tag="onesp_32")
            nc.vector.memset(onesp_32[:], 1.0)
            onesp = pp.tile([P, 1], BF16, tag="onesp")
            nc.vector.tensor_copy(onesp[:], onesp_32[:])
            ones_col128_f = pp.tile([1, P], F32, tag="ones_col128_f")
            nc.vector.memset(ones_col128_f[:], 1.0)
            ones_col128 = pp.tile([1, P], F32R, tag="ones_col128")
            nc.vector.tensor_copy(ones_col128[:], ones_col128_f[:])
            epsp = pp.tile([P, 1], F32, tag="epsp")
            nc.vector.memset(epsp[:], EPS)

            wqkv = []
            for c in range(DC):
                t = pp.tile([P, QKV_COLS], BF16, tag=f"wqkv{c}")
                nc.gpsimd.dma_start(t[:], wqkv_ext.ap()[c * P:(c + 1) * P, :])
                wqkv.append(t)
            if with_qkv_bias:
                qkv_bias = []
                for grp in range(6):
                    bg = pp.tile([P, 1], F32, tag=f"bqkv{grp}")
                    nc.sync.dma_start(bg[:],
                                      bqkv_ext.ap()[grp * P:(grp + 1) * P, :])
                    qkv_bias.append(bg)

            wo = []          # wout chunk c, half hf: [128, 512] bf16
            for c in range(DC):
                for hf in range(2):
                    t = pp.tile([P, 512], BF16, tag=f"wo{c}_{hf}")
                    nc.gpsimd.dma_start(
                        t[:], wout_ext.ap()[c * P:(c + 1) * P,
                                            hf * 512:(hf + 1) * 512])
                    wo.append(t)

            zselfA = pp.tile([4, P], F32, tag="zselfA")
            nc.gpsimd.dma_start(zselfA[:], zsel_ext.ap()[0:4, :])
            zselfB = pp.tile([4, P], F32, tag="zselfB")
            nc.gpsimd.dma_start(zselfB[:], zsel_ext.ap()[4:8, :])
            zselA = pp.tile([4, P], F32R, tag="zselA")
            nc.vector.tensor_copy(zselA[:], zselfA[:])
            zselB = pp.tile([4, P], F32R, tag="zselB")
            nc.vector.tensor_copy(zselB[:], zselfB[:])

            bout = pp.tile([1, D], F32R, tag="bout")
            nc.gpsimd.dma_start(bout[:], bout_ext.ap())
            bout_bc = pp.tile([P, D], F32, tag="bout_bc")
            for half in range(2):
                bb = ps_m.tile([P, 512], F32, tag="m", name=f"bbp_{half}")
                nc.tensor.matmul(bb[:], ones_col128[:],
                                 bout[0:1, half * 512:(half + 1) * 512],
                                 start=True, stop=True)
                nc.vector.tensor_copy(bout_bc[:, half * 512:(half + 1) * 512],
                                      bb[:])

            # persistent activations, per head-pair (partitions 0-63 = even
            # head of the pair, 64-127 = odd head)
            qT = [pp.tile([P, NB], BF16, tag=f"qT{p}", name=f"qT{p}") for p in range(2)]
            kT = [pp.tile([P, NB], BF16, tag=f"kT{p}", name=f"kT{p}") for p in range(2)]
            vaug = [pp.tile([P, MT * 130], BF16, tag=f"vaug{p}",
                            name=f"vaug{p}") for p in range(2)]

            # 8-core a2a, one per head-pair. Senders duplicate their slot
            # data into both batch-group slot ranges (j and j+4) so the
            # program stays core-uniform; receivers mask the wrong-batch
            # half via zsel4 (host-supplied, exact zeros).
            a2a_groups = [[0, 1, 2, 3, 4, 5, 6, 7]]
            a2a_in = [dram.tile([8, 130, 512], BF16, name=f"a2a_in{p}",
                                tag=f"a2a_in{p}") for p in range(2)]
            a2a_out = [dram.tile([8, 130, 512], BF16, name=f"a2a_out{p}",
                                 tag=f"a2a_out{p}") for p in range(2)]

            # ---- phase 1: per-block fused LN + transposes + QKV ----------
            def qkv_block(blk):
                xt = xsp.tile([P, 4, D], F32, tag="xt", name=f"xt_{blk}")
                nc.sync.dma_start(
                    xt[:],
                    x_ext.ap()[blk * 512:(blk + 1) * 512, :]
                    .rearrange("(a p) d -> p a d", p=P))

                xbt = xbp.tile([P, 4, D], BF16, tag="xbt", name=f"xb_{blk}")
                for t in range(4):
                    st = smp.tile([P, 2, 6], F32, tag="st", bufs=4,
                                  name=f"st_{blk}_{t}")
                    nc.vector.bn_stats(st[:, 0, :], xt[:, t, 0:512])
                    nc.vector.bn_stats(st[:, 1, :], xt[:, t, 512:1024])
                    mv = smp.tile([P, 2], F32, tag="mv", bufs=4,
                                  name=f"mv_{blk}_{t}")
                    nc.vector.bn_aggr(mv[:], st[:])
                    sd = smp.tile([P, 1], F32, tag="sd", bufs=4,
                                  name=f"sd_{blk}_{t}")
                    nc.scalar.activation(sd[:], mv[:, 1:2], AF.Sqrt,
                                         bias=epsp[:])
                    rstd = smp.tile([P, 1], F32, tag="rstd", bufs=4,
                                    name=f"rstd_{blk}_{t}")
                    nc.vector.reciprocal(rstd[:], sd[:])
                    nmr = smp.tile([P, 1], F32, tag="nmr", bufs=4,
                                   name=f"nmr_{blk}_{t}")
                    nc.vector.tensor_mul(nmr[:], mv[:, 0:1], rstd[:])
                    nc.vector.tensor_scalar(nmr[:], nmr[:], -1.0, None,
                                            OP.mult)
                    # normalized bf16 cast on GpSimd: xn = x*rstd - mu*rstd
                    nc.gpsimd.tensor_scalar(xbt[:, t, :], xt[:, t, :],
                                            rstd[:], nmr[:],
                                            OP.mult, OP.add)

                # x^T chunks via PE transposes (bf16)
                xts = xbp.tile([P, DC, 512], BF16, tag="xts",
                               name=f"xts_{blk}")
                for c in range(DC):
                    xtp = ps_m.tile([P, 512], BF16, tag="m",
                                    name=f"xtp_{blk}_{c}")
                    for t in range(4):
                        nc.tensor.transpose(
                            xtp[:, t * P:(t + 1) * P],
                            xbt[:, t, c * P:(c + 1) * P], identb[:])
                    nc.vector.tensor_copy(xts[:, c, :], xtp[:])

                # QKV: 6 groups of 128 cols (q/k/v x head-pair)
                vtb = [None, None]
                for p in range(2):
                    vtb[p] = xbp.tile([P, 512], BF16, tag=f"vtb{p}",
                                      name=f"vtb_{blk}_{p}")
                targets = [(qT[0], blk * 512), (qT[1], blk * 512),
                           (kT[0], blk * 512), (kT[1], blk * 512),
                           (vtb[0], 0), (vtb[1], 0)]
                for grp, (dst, col) in enumerate(targets):
                    acc = ps_q.tile([P, 512], F32, tag="q",
                                    name=f"qkv_{blk}_{grp}")
                    for c in range(DC):
                        nc.tensor.matmul(acc[:],
                                         wqkv[c][:, grp * P:(grp + 1) * P],
                                         xts[:, c, :],
                                         start=(c == 0), stop=(c == DC - 1))
                    nc.vector.tensor_copy(dst[:, col:col + 512], acc[:])
                    if with_qkv_bias:
                        nc.vector.tensor_scalar(dst[:, col:col + 512],
                                                dst[:, col:col + 512],
                                                qkv_bias[grp][:], None, OP.add)

                # v^T -> vaug (token-major v plus ones column per head)
                for p in range(2):
                    for t in range(4):
                        i = blk * 4 + t
                        vtp = ps_m.tile([P, P], BF16, tag="m",
                                        name=f"vtp_{blk}_{p}_{t}")
                        nc.tensor.transpose(vtp[:], vtb[p][:, t * P:(t + 1) * P],
                                            identb[:])
                        base = i * 130
                        nc.vector.tensor_copy(vaug[p][:, base:base + 64],
                                              vtp[:, 0:64])
                        nc.vector.tensor_copy(vaug[p][:, base + 65:base + 129],
                                              vtp[:, 64:128])
                        nc.vector.tensor_copy(vaug[p][:, base + 64:base + 65],
                                              onesp[:])
                        nc.vector.tensor_copy(vaug[p][:, base + 129:base + 130],
                                              onesp[:])

            for blk in range(NBLK):
                qkv_block(blk)

            # ---- phase 2: attention (pure S -> exp -> PV) ----------------
            def attention(p, hl, qb):
                # p: head-pair, hl: head within pair, qb: 1024-query block
                hp = hl * DH
                q0 = qb * 1024
                es = []
                for m in range(MT):
                    s = ps_s.tile([P, 1024], F32, tag="s",
                                  name=f"s_{p}_{hl}_{qb}_{m}")
                    for hf in range(2):
                        nc.tensor.matmul(
                            s[:, hf * 512:(hf + 1) * 512],
                            kT[p][hp:hp + DH, m * P:(m + 1) * P],
                            qT[p][hp:hp + DH, q0 + hf * 512:q0 + (hf + 1) * 512],
                            start=True, stop=True)
                    e = esp.tile([P, 1024], BF16, tag="e",
                                 name=f"e_{p}_{hl}_{qb}_{m}")
                    nc.scalar.activation(e[:], s[:], AF.Exp, bias=0.0,
                                         scale=SCALE)
                    es.append(e)
                for hf in range(2):
                    j = qb * 2 + hf
                    sa = ps_m.tile([65, 512], F32, tag="m",
                                   name=f"sa_{p}_{hl}_{qb}_{hf}")
                    for m in range(MT):
                        nc.tensor.matmul(
                            sa[:],
                            vaug[p][:, m * 130 + hl * 65: m * 130 + (hl + 1) * 65],
                            es[m][:, hf * 512:(hf + 1) * 512],
                            start=(m == 0), stop=(m == MT - 1))
                    sab = smp.tile([65, 512], BF16, tag="sab", bufs=2,
                                   name=f"sab_{p}_{hl}_{qb}_{hf}")
                    nc.vector.tensor_copy(sab[:], sa[:])
                    nc.sync.dma_start(
                        a2a_in[p][j, hl * 65:(hl + 1) * 65, :], sab[:])
                    nc.sync.dma_start(
                        a2a_in[p][j + 4, hl * 65:(hl + 1) * 65, :], sab[:])

            for p in range(2):
                for hl in range(2):
                    for qb in range(2):
                        attention(p, hl, qb)
                nc.gpsimd.collective_compute(
                    "AllToAll", OP.bypass,
                    replica_groups=a2a_groups,
                    ins=[a2a_in[p].opt()],
                    outs=[a2a_out[p].opt()],
                )

            # ---- phase 3: batched loads, mask+normalize, out-projection --
            # inner-dim chunk c (=128 rows) holds heads {2c, 2c+1}
            #   = (group peer j = c//2, pair pl = c%2, heads hl in {0,1})
            # Slot sets A (0:4) and B (4:8) carry batch 0 / batch 1 data;
            # zsel4 rows (A-hl0, A-hl1, B-hl0, B-hl1) are exact-zero masks
            # for the wrong batch, so xan = xanA*zbcA + xanB*zbcB selects
            # and normalizes in one pass.
            xanA = smp.tile([P, 4, 2, 512], BF16, tag="xanA", bufs=1,
                            name="xanA")
            xanB = smp.tile([P, 4, 2, 512], BF16, tag="xanB", bufs=1,
                            name="xanB")
            zall = smp.tile([4, 4, 2, 512], BF16, tag="zall", bufs=1,
                            name="zall")
            for pl in range(2):
                for hl in range(2):
                    nc.sync.dma_start(
                        xanA[hl * DH:(hl + 1) * DH, :, pl, :],
                        a2a_out[pl][0:4, hl * 65:hl * 65 + 64, :]
                        .rearrange("j p t -> p j t"))
                    nc.sync.dma_start(
                        xanB[hl * DH:(hl + 1) * DH, :, pl, :],
                        a2a_out[pl][4:8, hl * 65:hl * 65 + 64, :]
                        .rearrange("j p t -> p j t"))
                    nc.sync.dma_start(
                        zall[hl:hl + 1, :, pl, :],
                        a2a_out[pl][0:4, hl * 65 + 64:hl * 65 + 65, :]
                        .rearrange("j p t -> p j t"))
                    nc.sync.dma_start(
                        zall[2 + hl:3 + hl, :, pl, :],
                        a2a_out[pl][4:8, hl * 65 + 64:hl * 65 + 65, :]
                        .rearrange("j p t -> p j t"))
            xanAf = xanA[:].rearrange("p j q t -> p (j q) t")
            xanBf = xanB[:].rearrange("p j q t -> p (j q) t")
            zallf = zall[:].rearrange("p j q t -> p (j q) t")
            xanF = xanAf  # normalized+masked result overwrites xanA
            for c in range(DC):
                zrf = smp.tile([4, 512], F32R, tag="zrf", bufs=1,
                               name=f"zrf_{c}")
                nc.vector.tensor_copy(zrf[:], zallf[:, c, :])
                zi4 = smp.tile([4, 512], F32R, tag="zi4", bufs=1,
                               name=f"zi_{c}")
                with nc.allow_low_precision(reason="f32r is full fp32 bits"):
                    nc.vector.reciprocal(zi4[:], zrf[:])
                zbcA = ps_m.tile([P, 512], F32, tag="m", name=f"zbcA_{c}")
                nc.tensor.matmul(zbcA[:], zselA[:], zi4[:],
                                 start=True, stop=True)
                zbcB = ps_m.tile([P, 512], F32, tag="m", name=f"zbcB_{c}")
                nc.tensor.matmul(zbcB[:], zselB[:], zi4[:],
                                 start=True, stop=True)
                nc.vector.tensor_mul(xanF[:, c, :], zbcA[:], xanAf[:, c, :])
                tmpB = smp.tile([P, 512], BF16, tag="tmpB", bufs=1,
                                name=f"tmpB_{c}")
                nc.vector.tensor_mul(tmpB[:], zbcB[:], xanBf[:, c, :])
                nc.vector.tensor_add(xanF[:, c, :], xanF[:, c, :], tmpB[:])

            for half in range(2):
                ot = smp.tile([P, 4, 512], F32, tag="ot", bufs=1,
                              name=f"ot_{half}")
                for t in range(4):
                    acc = ps_q.tile([P, 512], F32, tag="q",
                                    name=f"op_{t}_{half}")
                    for c in range(DC):
                        nc.tensor.matmul(acc[:],
                                         xanF[:, c, t * P:(t + 1) * P],
                                         wo[c * 2 + half][:],
                                         start=(c == 0), stop=(c == DC - 1))
                    nc.vector.tensor_add(ot[:, t, :], acc[:],
                                         bout_bc[:, half * 512:(half + 1) * 512])
                nc.sync.dma_start(
                    out_ext.ap()[:, half * 512:(half + 1) * 512]
                    .rearrange("(a p) d -> p a d", p=P),
                    ot[:])

    nc.compile()
    return nc


_NC_CACHE = {}
_last_in_maps = None


def kernel(x, gamma, beta, w_qkv, w_out, b_out):
    x = np.ascontiguousarray(np.asarray(x, dtype=np.float32).reshape(NT, D))
    gamma = np.asarray(gamma, dtype=np.float32)
    beta = np.asarray(beta, dtype=np.float32)
    w_qkv = np.asarray(w_qkv, dtype=np.float32)
    w_out = np.ascontiguousarray(np.asarray(w_out, dtype=np.float32))
    b_out = np.asarray(b_out, dtype=np.float32)

    w_eff = gamma[:, None] * w_qkv            # [1024, 3072]
    b_eff = beta @ w_qkv                      # [3072]
    with_bias = bool(np.any(b_eff != 0.0))

    if with_bias not in _NC_CACHE:
        _NC_CACHE[with_bias] = _build(with_bias)
    nc = _NC_CACHE[with_bias]

    w_bf = w_eff.astype(ml_dtypes.bfloat16)
    wout_bf = w_out.astype(ml_dtypes.bfloat16)
    ident = np.eye(P, dtype=np.float32)
    # per-core slot-set mask + z-broadcast selector (built per core below)

    in_maps = []
    for g in range(8):
        # qkv cols: 6 groups of 128 = (q,k,v) x head-pair {0,1} of this core
        heads = [4 * (g % 4) + i for i in range(4)]
        cols = []
        for part in range(3):
            for pair in range(2):
                for hh in range(2):
                    h = heads[2 * pair + hh]
                    c0 = part * D + h * DH
                    cols.append(np.arange(c0, c0 + DH))
        cols = np.concatenate(cols)
        xg = x[(g // 4) * N:(g // 4 + 1) * N, :]
        zsel = np.zeros((8, P), dtype=np.float32)
        mA = 1.0 if g < 4 else 0.0
        mB = 1.0 - mA
        zsel[0, 0:DH] = mA      # A set, hl=0 rows
        zsel[1, DH:P] = mA      # A set, hl=1 rows
        zsel[6, 0:DH] = mB      # B set, hl=0 rows
        zsel[7, DH:P] = mB      # B set, hl=1 rows
        in_maps.append({
            "x": np.ascontiguousarray(xg),
            "wqkv": np.ascontiguousarray(w_bf[:, cols]),
            "bqkv": np.ascontiguousarray(b_eff[cols][:, None]),
            "wout": wout_bf,
            "bout": np.ascontiguousarray(b_out[None, :]),
            "ident": ident,
            "zsel": zsel,
        })

    global _last_in_maps
    _last_in_maps = in_maps
    res = run_bass_kernel_spmd(nc, in_maps, core_ids=list(range(8)))
    out = np.empty((NT, D), dtype=np.float32)
    for g in range(8):
        out[g * TOK_OUT:(g + 1) * TOK_OUT, :] = res.results[g]["out"]
    return out.reshape(B, N, D)
